# revision 23
# baseline (speedup 1.0000x reference)
"""Trainium2 Bass kernel for DendriticANN (dense_mlp).

Reference computation (fp32):
    h = lrelu(x @ W_in.T + b_in)                        # [B, H]
    for l in 0..L-1:
        dend = lrelu(einsum('bh,ndh->bnd', h, Wd[l]))   # [B, H, D]
        soma = lrelu(einsum('bnd,nd->bn', dend, sd[l])) # [B, H]
        h = lrelu(soma)
    out = h @ W_out.T + b_out                           # [B, OUT]

Strategy: tensor-parallel over the H neuron axis across 8 NeuronCores,
with a 2-way batch split to software-pipeline the inter-layer AllGathers
under compute. Activations live transposed on-chip (hT = [H part, B
free]) so every matmul contracts over the partition dim:

  - all matmul operands in bf16 (bf16 rhs streams ~2 cols/cycle on the
    PE; psum accumulation stays fp32)
  - Wd (bf16, 16.8 MB/core) is SBUF-resident, loaded once per NEFF.
    One-time loads go on the ACT HWDGE ring (small tensors + wd layer
    0 in 4 chunks, so layer-0 matmuls start as soon as tiles land) and
    the SWDGE/Pool queue (wd layers 1-3, descriptors emitted in ~us and
    drained in the background) -- never on the SP ring, which carries
    the latency-critical per-pass DMAs
  - input layer: H-sharded on every core for the full batch (8 matmuls
    N=512), one AllGather of [H, B]
  - hidden layer per core per batch-half: for each of 16
    (neuron,dendrite) 128-row tiles: dend^T via 8 accumulating K=128
    matmuls, lrelu on ScalarE (alpha=.01), soma via blockdiag(sd)
    matmul (PE does the D-sum; 4 tiles packed into one PSUM bank via
    32-wide PE column groups so they stream concurrently), then
    lrelu(lrelu(.)) = Prelu alpha=1e-4
  - somaS partition 32j+p slot G -> agin row 32j+4p+G ("storage
    order": each per-j DMA is 8 fat contiguous descriptors); every
    weight tensor contracting over h is pre-permuted on the host to
    match, so the gathered rows are consumed as-is
  - output layer sharded over OUT rows: outT_c = W_out_c @ hT + b_out_c
    (host concatenates the 8 shards; final transpose on host)
"""

import numpy as np

import concourse.bass as bass
import concourse.mybir as mybir
import concourse.tile as tile
from concourse import bacc
from concourse.bass_utils import run_bass_kernel_spmd

# Problem constants (hardcoded per harness contract)
B, IN, H, OUT, L, D = 512, 1024, 1024, 1000, 4, 16
N_CORES = 8
HS = H // N_CORES           # 128 neurons per core
OS = OUT // N_CORES         # 125 output rows per core
KT = H // 128               # 8 k-tiles over the contraction dim
NDT = HS * D // 128         # 16 (neuron,dendrite) tiles of 128 per core
N8 = 128 // D               # 8 neurons per nd-tile
NH = 2                      # batch chunks for gather/compute pipelining


def _bounds(nh):
    cuts = [round(B * i / nh) for i in range(nh + 1)]
    return [(cuts[i], cuts[i + 1]) for i in range(nh)]

AF = mybir.ActivationFunctionType
F32 = mybir.dt.float32
BF16 = mybir.dt.bfloat16


def build_module(mm_dt=None, wd_bufs=None, reps=1, ablate=(), nh=None):
    """Build + compile the SPMD Bass module. Returns nc.

    reps > 1 unrolls the whole pipeline R times inside one NEFF — used by
    test.py to measure steady-state per-iteration device time via the
    slope between rep counts (no NTFF profiling available under axon).

    ablate: subset of {"noag", "extra_ag"} — timing-only variants
    ("noag" produces WRONG results but isolates collective cost;
    "extra_ag" adds one dummy AllGather per pass whose output nothing
    consumes, to probe whether collectives act as global barriers).
    """
    del mm_dt, wd_bufs  # vestigial (bf16-only, weights SBUF-resident)
    if nh is None:
        nh = NH
    bounds = _bounds(nh)
    ablate = set(ablate)
    sdt = BF16
    nc = bacc.Bacc("TRN2", target_bir_lowering=False, debug=False,
                   num_devices=N_CORES)

    # ---- DRAM I/O (per-core shards, host-prepared layouts) ----
    xT_d = nc.dram_tensor("xT", [128, KT, B], sdt, kind="ExternalInput").ap()
    winT_d = nc.dram_tensor("winT", [128, KT, H], sdt,
                            kind="ExternalInput").ap()
    bin_d = nc.dram_tensor("b_in", [128, KT], F32, kind="ExternalInput").ap()
    # wd: partition-major contiguous — [l, k, (t, kt, m, d)]
    wd_d = nc.dram_tensor("wdT", [L, 128, NDT * KT * 128], sdt,
                          kind="ExternalInput").ap()
    sdb_d = nc.dram_tensor("sdb", [128, L * NDT * N8], sdt,
                           kind="ExternalInput").ap()
    woutT_d = nc.dram_tensor("woutT", [128, KT, OS], sdt,
                             kind="ExternalInput").ap()
    bout_d = nc.dram_tensor("b_out", [OS, 1], F32, kind="ExternalInput").ap()
    outT_d = nc.dram_tensor("outT", [OS, B], F32, kind="ExternalOutput").ap()

    rg = [list(range(N_CORES))]

    with tile.TileContext(nc) as tc:
        with (
            tc.tile_pool(name="const", bufs=1) as cpool,
            tc.tile_pool(name="h", bufs=4) as hpool,
            tc.tile_pool(name="s1p", bufs=6) as s1pool,
            tc.tile_pool(name="soma", bufs=3) as spool,
            tc.tile_pool(name="outp", bufs=2) as opool,
            tc.tile_pool(name="psd", bufs=4, space="PSUM") as ppd,
            tc.tile_pool(name="pss", bufs=2, space="PSUM") as pps,
            tc.tile_pool(name="dram", bufs=3, space="DRAM") as dpool,
        ):
            # ---- persistent loads: small tensors + wd layer 0 on the ACT
            # HWDGE ring (drains in issue order, independently of the SP
            # ring); wd layers 1-3 via SWDGE (background drain) ----
            xT = cpool.tile([128, KT, B], sdt, name="xT_sb")
            nc.scalar.dma_start(xT[:], xT_d[:])
            winT = cpool.tile([128, KT, H], sdt, name="winT_sb")
            nc.scalar.dma_start(winT[:], winT_d[:])
            b_in = cpool.tile([128, KT], F32, name="bin_sb")
            nc.scalar.dma_start(b_in[:], bin_d[:])
            sdb = cpool.tile([128, L * NDT * N8], sdt, name="sdb_sb")
            nc.scalar.dma_start(sdb[:], sdb_d[:])
            wd_sb = []
            for l in range(L):
                w = cpool.tile([128, NDT, KT * 128], sdt, name=f"wd_sb_l{l}")
                wd_sb.append(w)
            TC0 = 4                 # tiles per chunk for the layer-0 load
            for c in range(NDT // TC0):
                nc.scalar.dma_start(
                    wd_sb[0][:, c * TC0:(c + 1) * TC0, :],
                    wd_d[0, :, c * TC0 * KT * 128:(c + 1) * TC0 * KT * 128])
            woutT = cpool.tile([128, KT, OS], sdt, name="woutT_sb")
            nc.scalar.dma_start(woutT[:], woutT_d[:])
            b_out = cpool.tile([OS, 1], F32, name="bout_sb")
            nc.scalar.dma_start(b_out[:], bout_d[:])
            for l in range(1, L):
                nc.gpsimd.dma_start(wd_sb[l][:], wd_d[l])

            def gather(agin, l, u, bh):
                """AllGather [128,bh] core shards -> full hT [128,KT,bh].

                Rows travel in "storage order" q = 32j+4p+G carrying
                neuron n(q) = 32G+8j+p — the order the somaS per-j DMA
                scan produces. All weight tensors that contract over h
                have their k-axis pre-permuted on the host to match, so
                no on-device shuffle is ever needed.
                """
                hT = hpool.tile([128, KT, bh], sdt, tag="hT",
                                name=f"hT_l{l}_u{u}")
                if "noag" in ablate:
                    for kt in range(KT):
                        nc.gpsimd.dma_start(hT[:, kt, :], agin[:])
                    return hT
                agout = dpool.tile([H, bh], sdt, addr_space="Shared",
                                   tag="agout", name=f"agout_l{l}_u{u}")
                nc.gpsimd.collective_compute(
                    "AllGather",
                    mybir.AluOpType.bypass,
                    replica_groups=rg,
                    ins=[agin[:].opt()],
                    outs=[agout[:].opt()],
                )
                # hT load on the Pool/SWDGE queue: it waits on this AG's
                # completion, which on the in-order Pool queue only delays
                # the NEXT collective trigger — and that one is gated on
                # the serial collective engine anyway.
                gv = agout[:].rearrange("(kt k) b -> k kt b", k=128)
                nc.gpsimd.dma_start(hT[:], gv)
                return hT

            def one_pass():
                # ---- input layer: redundant full-H compute on every core
                # (~6us of extra PE work replaces a ~9us collective
                # barrier, writes hT tiles directly, and needs no DMA) ----
                hT = [hpool.tile([128, KT, c1 - c0], sdt, tag="hT",
                                 name=f"hT_in_u{u}")
                      for u, (c0, c1) in enumerate(bounds)]
                for mt in range(KT):
                    ps0 = ppd.tile([128, B], F32, tag="pdw", bufs=2,
                                   name=f"ps0_m{mt}")
                    for kt in range(KT):
                        nc.tensor.matmul(
                            ps0[:], winT[:, kt, mt * 128:(mt + 1) * 128],
                            xT[:, kt, :],
                            start=(kt == 0), stop=(kt == KT - 1))
                    for u, (c0, c1) in enumerate(bounds):
                        nc.scalar.activation(hT[u][:, mt, :], ps0[:, c0:c1],
                                             AF.Lrelu,
                                             bias=b_in[:, mt:mt + 1],
                                             alpha=0.01)

                # ---- hidden layers, pipelined over batch chunks ----
                for l in range(L):
                    for u in range(nh):
                        bh = bounds[u][1] - bounds[u][0]
                        somaS = spool.tile([128, NDT // 4, bh], sdt,
                                           tag="soma", name=f"somaS_l{l}_u{u}")

                        def emit_somas(G, s1s):
                            # 4 soma matmuls (M=8) packed into one PSUM bank
                            # via 32-wide PE column groups -> they stream
                            # concurrently instead of serially. Emitted one
                            # dend tile AFTER their s1 inputs complete so the
                            # PE never waits on ScalarE latency.
                            pssG = pps.tile([128, bh], F32, tag="ps",
                                            name=f"ps_l{l}_u{u}_G{G}")
                            for j in range(4):
                                t = 4 * G + j
                                off = (l * NDT + t) * N8
                                nc.tensor.matmul(
                                    pssG[32 * j:32 * j + N8, :],
                                    sdb[:, off:off + N8], s1s[j][:],
                                    start=True, stop=True,
                                    tile_position=(0, 32 * j),
                                    skip_group_check=True)
                            # h' = lrelu(lrelu(soma)) = lrelu_{1e-4}(soma).
                            # Prelu gets its own table -> single fused op.
                            # Partitions 32j+8..32j+31 hold garbage (never
                            # matmul-written) but are never read downstream.
                            nc.scalar.activation(somaS[:, G, :], pssG[:],
                                                 AF.Prelu, alpha=1e-4)

                        s1s_all = []
                        for t in range(NDT):
                            psd = ppd.tile([128, bh], F32, tag="pd",
                                           name=f"pd_l{l}_u{u}_t{t}")
                            for kt in range(KT):
                                nc.tensor.matmul(
                                    psd[:],
                                    wd_sb[l][:, t, kt * 128:(kt + 1) * 128],
                                    hT[u][:, kt, :],
                                    start=(kt == 0), stop=(kt == KT - 1),
                                )
                            s1 = s1pool.tile([128, bh], sdt, tag="s1",
                                             name=f"s1_l{l}_u{u}_t{t}")
                            nc.scalar.activation(s1[:], psd[:], AF.Lrelu,
                                                 alpha=0.01)
                            s1s_all.append(s1)
                            if t % 4 == 0 and t >= 4:
                                G = t // 4 - 1
                                emit_somas(G, s1s_all[4 * G:4 * G + 4])
                        emit_somas(NDT // 4 - 1, s1s_all[-4:])
                        # somaS partition 32j+p, free slot G -> agin row
                        # q = 32j+4p+G (storage order; 8 fat descriptors
                        # per j-DMA)
                        agin = dpool.tile([HS, bh], sdt, tag="agin",
                                          name=f"agin_l{l}_u{u}")
                        for j in range(4):
                            sout = agin[32 * j:32 * j + 32, :].rearrange(
                                "(p G) b -> p G b", p=N8)
                            nc.sync.dma_start(
                                sout, somaS[32 * j:32 * j + N8, :, :])
                        hT[u] = gather(agin, l, u, bh)

                # ---- output layer (OUT-sharded) ----
                for u in range(nh):
                    c0, c1 = bounds[u]
                    bh = c1 - c0
                    pso = ppd.tile([OS, bh], F32, tag="pd", name=f"pso_u{u}")
                    for kt in range(KT):
                        nc.tensor.matmul(pso[:], woutT[:, kt, :],
                                         hT[u][:, kt, :],
                                         start=(kt == 0), stop=(kt == KT - 1))
                    out_sb = opool.tile([OS, bh], F32, tag="out",
                                        name=f"out_sb_u{u}")
                    nc.scalar.activation(out_sb[:], pso[:], AF.Identity,
                                         bias=b_out[:])
                    nc.sync.dma_start(outT_d[:, c0:c1], out_sb[:])

            for _rep in range(reps):
                one_pass()

    nc.compile()
    return nc


def make_in_maps(x, W_in, b_in, Wd, sd, W_out, b_out, mm_dt=None):
    """Host-side sharding/layout prep. Returns per-core input dicts."""
    del mm_dt
    import ml_dtypes
    ndt = np.dtype(ml_dtypes.bfloat16)
    f32 = np.float32
    x = np.asarray(x, f32)
    W_in = np.asarray(W_in, f32)
    b_in = np.asarray(b_in, f32)
    Wd = np.asarray(Wd, f32)
    sd = np.asarray(sd, f32)
    W_out = np.asarray(W_out, f32)
    b_out = np.asarray(b_out, f32)

    # xT: [k, kt, b] (shared by all cores)
    xT = np.ascontiguousarray(
        x.reshape(B, KT, 128).transpose(2, 1, 0)).astype(ndt)

    # h travels in "storage order": gathered row q (within a 128 block)
    # carries neuron n(q) = 32G+8j+p for q = 32j+4p+G — the order the
    # on-device somaS->agin per-j DMA scan produces. Pre-permute every
    # weight axis that produces or contracts h accordingly.
    qv = np.arange(128)
    perm = 32 * (qv % 4) + 8 * (qv // 32) + (qv % 32) // 4   # n(q)

    # input layer: winT column q of block mt = neuron mt*128+perm[q] over
    # the FULL H, so the redundantly computed hT sits in storage order
    permH = (np.arange(H) // 128) * 128 + np.tile(perm, KT)
    WiP = W_in[permH, :]                                       # [H, IN]
    winT = np.ascontiguousarray(
        WiP.reshape(H, KT, 128).transpose(2, 1, 0)).astype(ndt)
    binP = np.ascontiguousarray(b_in[permH].reshape(KT, 128).T)

    in_maps = []
    for c in range(N_CORES):
        Wd_c = Wd[:, c * HS:(c + 1) * HS, :, :]                # [L, 128, D, H]
        # [l, t, m, d, kt, k] -> [l, k, t, kt, m, d]: partition-major so the
        # device load is one contiguous 32 KB read per partition per layer;
        # contraction rows permuted to storage order (k-axis q <- perm[q])
        wdT = np.ascontiguousarray(
            Wd_c.reshape(L, NDT, N8, D, KT, 128)[..., perm]
            .transpose(0, 5, 1, 4, 2, 3)
        ).reshape(L, 128, NDT * KT * 128).astype(ndt)

        sd_c = sd[:, c * HS:(c + 1) * HS, :]                   # [L, 128, D]
        sd_r = sd_c.reshape(L, NDT, N8, D)                     # [l, t, m, d]
        sdb = np.zeros((128, L, NDT, N8), f32)
        for m in range(N8):
            # partition nd = m*D + d gets sd of neuron m in each tile
            sdb[m * D:(m + 1) * D, :, :, m] = sd_r[:, :, m, :].transpose(2, 0, 1)
        sdb = np.ascontiguousarray(sdb.reshape(128, L * NDT * N8)).astype(ndt)

        Wo = W_out[c * OS:(c + 1) * OS, :]                     # [125, H]
        woutT = np.ascontiguousarray(
            Wo.reshape(OS, KT, 128)[..., perm].transpose(2, 1, 0)).astype(ndt)
        bout_c = np.ascontiguousarray(b_out[c * OS:(c + 1) * OS, None])

        in_maps.append({
            "xT": xT,
            "winT": winT,
            "b_in": binP,
            "wdT": wdT,
            "sdb": sdb,
            "woutT": woutT,
            "b_out": bout_c,
        })
    return in_maps


_CACHE = {}


def get_module(mm_dt=None, wd_bufs=None):
    key = ("v4",)
    if key not in _CACHE:
        _CACHE[key] = build_module()
    return _CACHE[key]


def kernel(x, W_in, b_in, Wd, sd, W_out, b_out):
    """Full-input -> full-output entry point (harness contract)."""
    nc = get_module()
    in_maps = make_in_maps(x, W_in, b_in, Wd, sd, W_out, b_out)
    res = run_bass_kernel_spmd(nc, in_maps, core_ids=list(range(N_CORES)))
    out = np.concatenate([res.results[c]["outT"].T for c in range(N_CORES)],
                         axis=1)
    return np.ascontiguousarray(out.astype(np.float32))


# revision 27
# speedup vs baseline: 1.0338x; 1.0338x over previous
"""Trainium2 Bass kernel for DendriticANN (dense_mlp).

Reference computation (fp32):
    h = lrelu(x @ W_in.T + b_in)                        # [B, H]
    for l in 0..L-1:
        dend = lrelu(einsum('bh,ndh->bnd', h, Wd[l]))   # [B, H, D]
        soma = lrelu(einsum('bnd,nd->bn', dend, sd[l])) # [B, H]
        h = lrelu(soma)
    out = h @ W_out.T + b_out                           # [B, OUT]

Strategy: tensor-parallel over the H neuron axis across 8 NeuronCores,
with a 2-way batch split to software-pipeline the inter-layer AllGathers
under compute. Activations live transposed on-chip (hT = [H part, B
free]) so every matmul contracts over the partition dim:

  - all matmul operands in bf16 (bf16 rhs streams ~2 cols/cycle on the
    PE; psum accumulation stays fp32)
  - Wd (bf16, 16.8 MB/core) is SBUF-resident, loaded once per NEFF.
    One-time loads go on the ACT HWDGE ring (small tensors + wd layer
    0 in 4 chunks, so layer-0 matmuls start as soon as tiles land) and
    the SWDGE/Pool queue (wd layers 1-3, descriptors emitted in ~us and
    drained in the background) -- never on the SP ring, which carries
    the latency-critical per-pass DMAs
  - input layer: computed redundantly over the FULL H on every core
    (64 matmuls N=512, ~6us of PE), writing the hT tiles directly —
    this replaces an input AllGather, and collectives here cost ~9us
    of exposed time each no matter how they are overlapped (measured:
    slope = compute + 9.2us x n_collectives, and a dummy AllGather
    with an unconsumed output still costs ~7us)
  - hidden layer per core per batch-half: for each of 16
    (neuron,dendrite) 128-row tiles: dend^T via 8 accumulating K=128
    matmuls, lrelu on ScalarE (alpha=.01), soma via blockdiag(sd)
    matmul (PE does the D-sum; 4 tiles packed into one PSUM bank via
    32-wide PE column groups so they stream concurrently), then
    lrelu(lrelu(.)) = Prelu alpha=1e-4
  - somaS partition 32j+p slot G -> agin row 32j+4p+G ("storage
    order": each per-j DMA is 8 fat contiguous descriptors); every
    weight tensor contracting over h is pre-permuted on the host to
    match, so the gathered rows are consumed as-is
  - output layer sharded over OUT rows: outT_c = W_out_c @ hT + b_out_c
    (host concatenates the 8 shards; final transpose on host)
"""

import numpy as np

import concourse.bass as bass
import concourse.mybir as mybir
import concourse.tile as tile
from concourse import bacc
from concourse.bass_utils import run_bass_kernel_spmd

# Problem constants (hardcoded per harness contract)
B, IN, H, OUT, L, D = 512, 1024, 1024, 1000, 4, 16
N_CORES = 8
HS = H // N_CORES           # 128 neurons per core
OS = OUT // N_CORES         # 125 output rows per core
KT = H // 128               # 8 k-tiles over the contraction dim
NDT = HS * D // 128         # 16 (neuron,dendrite) tiles of 128 per core
N8 = 128 // D               # 8 neurons per nd-tile
NH = 2                      # batch chunks for gather/compute pipelining


def _bounds(nh):
    cuts = [round(B * i / nh) for i in range(nh + 1)]
    return [(cuts[i], cuts[i + 1]) for i in range(nh)]

AF = mybir.ActivationFunctionType
F32 = mybir.dt.float32
BF16 = mybir.dt.bfloat16


def build_module(mm_dt=None, wd_bufs=None, reps=1, ablate=(), nh=None):
    """Build + compile the SPMD Bass module. Returns nc.

    reps > 1 unrolls the whole pipeline R times inside one NEFF — used by
    test.py to measure steady-state per-iteration device time via the
    slope between rep counts (no NTFF profiling available under axon).

    ablate: subset of {"noag", "extra_ag"} — timing-only variants
    ("noag" produces WRONG results but isolates collective cost;
    "extra_ag" adds one dummy AllGather per pass whose output nothing
    consumes, to probe whether collectives act as global barriers).
    """
    del mm_dt, wd_bufs  # vestigial (bf16-only, weights SBUF-resident)
    if nh is None:
        nh = NH
    bounds = _bounds(nh)
    ablate = set(ablate)
    sdt = BF16
    nc = bacc.Bacc("TRN2", target_bir_lowering=False, debug=False,
                   num_devices=N_CORES)

    # ---- DRAM I/O (per-core shards, host-prepared layouts) ----
    xT_d = nc.dram_tensor("xT", [128, KT, B], sdt, kind="ExternalInput").ap()
    winT_d = nc.dram_tensor("winT", [128, KT, H], sdt,
                            kind="ExternalInput").ap()
    bin_d = nc.dram_tensor("b_in", [128, KT], F32, kind="ExternalInput").ap()
    # wd: partition-major contiguous — [l, k, (t, kt, m, d)]
    wd_d = nc.dram_tensor("wdT", [L, 128, NDT * KT * 128], sdt,
                          kind="ExternalInput").ap()
    sdb_d = nc.dram_tensor("sdb", [128, L * NDT * N8], sdt,
                           kind="ExternalInput").ap()
    woutT_d = nc.dram_tensor("woutT", [128, KT, OS], sdt,
                             kind="ExternalInput").ap()
    bout_d = nc.dram_tensor("b_out", [OS, 1], F32, kind="ExternalInput").ap()
    outT_d = nc.dram_tensor("outT", [OS, B], F32, kind="ExternalOutput").ap()

    rg = [list(range(N_CORES))]

    with tile.TileContext(nc) as tc:
        with (
            tc.tile_pool(name="const", bufs=1) as cpool,
            tc.tile_pool(name="h", bufs=4) as hpool,
            tc.tile_pool(name="s1p", bufs=6) as s1pool,
            tc.tile_pool(name="soma", bufs=3) as spool,
            tc.tile_pool(name="outp", bufs=2) as opool,
            tc.tile_pool(name="psd", bufs=4, space="PSUM") as ppd,
            tc.tile_pool(name="pss", bufs=2, space="PSUM") as pps,
            tc.tile_pool(name="dram", bufs=3, space="DRAM") as dpool,
        ):
            # ---- persistent loads: small tensors + wd layer 0 on the ACT
            # HWDGE ring (drains in issue order, independently of the SP
            # ring); wd layers 1-3 via SWDGE (background drain) ----
            xT = cpool.tile([128, KT, B], sdt, name="xT_sb")
            nc.scalar.dma_start(xT[:], xT_d[:])
            winT = cpool.tile([128, KT, H], sdt, name="winT_sb")
            nc.scalar.dma_start(winT[:], winT_d[:])
            b_in = cpool.tile([128, KT], F32, name="bin_sb")
            nc.scalar.dma_start(b_in[:], bin_d[:])
            # warm the ACT function-table sets (Lrelu/Prelu/Identity, same
            # alphas as the real ops) while the weight DMAs drain — the
            # ~2.7us-per-set PSEUDO_LOAD_ACT_FUNC_SET otherwise lands on
            # the cold critical path at the first input-layer activation
            warm = cpool.tile([128, 1], F32, name="act_warm")
            nc.scalar.activation(warm[:], b_in[:, 0:1], AF.Lrelu, alpha=0.01)
            nc.scalar.activation(warm[:], b_in[:, 0:1], AF.Prelu, alpha=1e-4)
            nc.scalar.activation(warm[:], b_in[:, 0:1], AF.Identity)
            sdb = cpool.tile([128, L * NDT * N8], sdt, name="sdb_sb")
            nc.scalar.dma_start(sdb[:], sdb_d[:])
            wd_sb = []
            for l in range(L):
                w = cpool.tile([128, NDT, KT * 128], sdt, name=f"wd_sb_l{l}")
                wd_sb.append(w)
            TC0 = 4                 # tiles per chunk for the layer-0 load
            for c in range(NDT // TC0):
                nc.scalar.dma_start(
                    wd_sb[0][:, c * TC0:(c + 1) * TC0, :],
                    wd_d[0, :, c * TC0 * KT * 128:(c + 1) * TC0 * KT * 128])
            woutT = cpool.tile([128, KT, OS], sdt, name="woutT_sb")
            nc.scalar.dma_start(woutT[:], woutT_d[:])
            b_out = cpool.tile([OS, 1], F32, name="bout_sb")
            nc.scalar.dma_start(b_out[:], bout_d[:])
            for l in range(1, L):
                nc.gpsimd.dma_start(wd_sb[l][:], wd_d[l])

            def gather(agin, l, u, bh):
                """AllGather [128,bh] core shards -> full hT [128,KT,bh].

                Rows travel in "storage order" q = 32j+4p+G carrying
                neuron n(q) = 32G+8j+p — the order the somaS per-j DMA
                scan produces. All weight tensors that contract over h
                have their k-axis pre-permuted on the host to match, so
                no on-device shuffle is ever needed.
                """
                hT = hpool.tile([128, KT, bh], sdt, tag="hT",
                                name=f"hT_l{l}_u{u}")
                if "noag" in ablate:
                    for kt in range(KT):
                        nc.gpsimd.dma_start(hT[:, kt, :], agin[:])
                    return hT
                agout = dpool.tile([H, bh], sdt, addr_space="Shared",
                                   tag="agout", name=f"agout_l{l}_u{u}")
                nc.gpsimd.collective_compute(
                    "AllGather",
                    mybir.AluOpType.bypass,
                    replica_groups=rg,
                    ins=[agin[:].opt()],
                    outs=[agout[:].opt()],
                )
                # hT load on the Pool/SWDGE queue: it waits on this AG's
                # completion, which on the in-order Pool queue only delays
                # the NEXT collective trigger — and that one is gated on
                # the serial collective engine anyway.
                gv = agout[:].rearrange("(kt k) b -> k kt b", k=128)
                nc.gpsimd.dma_start(hT[:], gv)
                return hT

            def one_pass():
                # ---- input layer: redundant full-H compute on every core
                # (~6us of extra PE work replaces a ~9us collective
                # barrier, writes hT tiles directly, and needs no DMA) ----
                hT = [hpool.tile([128, KT, c1 - c0], sdt, tag="hT",
                                 name=f"hT_in_u{u}")
                      for u, (c0, c1) in enumerate(bounds)]
                for mt in range(KT):
                    ps0 = ppd.tile([128, B], F32, tag="pdw", bufs=2,
                                   name=f"ps0_m{mt}")
                    for kt in range(KT):
                        nc.tensor.matmul(
                            ps0[:], winT[:, kt, mt * 128:(mt + 1) * 128],
                            xT[:, kt, :],
                            start=(kt == 0), stop=(kt == KT - 1))
                    for u, (c0, c1) in enumerate(bounds):
                        nc.scalar.activation(hT[u][:, mt, :], ps0[:, c0:c1],
                                             AF.Lrelu,
                                             bias=b_in[:, mt:mt + 1],
                                             alpha=0.01)

                # ---- hidden layers, pipelined over batch chunks ----
                for l in range(L):
                    for u in range(nh):
                        bh = bounds[u][1] - bounds[u][0]
                        somaS = spool.tile([128, NDT // 4, bh], sdt,
                                           tag="soma", name=f"somaS_l{l}_u{u}")

                        def emit_somas(G, s1s):
                            # 4 soma matmuls (M=8) packed into one PSUM bank
                            # via 32-wide PE column groups -> they stream
                            # concurrently instead of serially. Emitted one
                            # dend tile AFTER their s1 inputs complete so the
                            # PE never waits on ScalarE latency.
                            pssG = pps.tile([128, bh], F32, tag="ps",
                                            name=f"ps_l{l}_u{u}_G{G}")
                            for j in range(4):
                                t = 4 * G + j
                                off = (l * NDT + t) * N8
                                nc.tensor.matmul(
                                    pssG[32 * j:32 * j + N8, :],
                                    sdb[:, off:off + N8], s1s[j][:],
                                    start=True, stop=True,
                                    tile_position=(0, 32 * j),
                                    skip_group_check=True)
                            # h' = lrelu(lrelu(soma)) = lrelu_{1e-4}(soma).
                            # Prelu gets its own table -> single fused op.
                            # Partitions 32j+8..32j+31 hold garbage (never
                            # matmul-written) but are never read downstream.
                            nc.scalar.activation(somaS[:, G, :], pssG[:],
                                                 AF.Prelu, alpha=1e-4)

                        s1s_all = []
                        for t in range(NDT):
                            psd = ppd.tile([128, bh], F32, tag="pd",
                                           name=f"pd_l{l}_u{u}_t{t}")
                            for kt in range(KT):
                                nc.tensor.matmul(
                                    psd[:],
                                    wd_sb[l][:, t, kt * 128:(kt + 1) * 128],
                                    hT[u][:, kt, :],
                                    start=(kt == 0), stop=(kt == KT - 1),
                                )
                            s1 = s1pool.tile([128, bh], sdt, tag="s1",
                                             name=f"s1_l{l}_u{u}_t{t}")
                            nc.scalar.activation(s1[:], psd[:], AF.Lrelu,
                                                 alpha=0.01)
                            s1s_all.append(s1)
                            if t % 4 == 0 and t >= 4:
                                G = t // 4 - 1
                                emit_somas(G, s1s_all[4 * G:4 * G + 4])
                        emit_somas(NDT // 4 - 1, s1s_all[-4:])
                        # somaS partition 32j+p, free slot G -> agin row
                        # q = 32j+4p+G (storage order; 8 fat descriptors
                        # per j-DMA)
                        agin = dpool.tile([HS, bh], sdt, tag="agin",
                                          name=f"agin_l{l}_u{u}")
                        for j in range(4):
                            sout = agin[32 * j:32 * j + 32, :].rearrange(
                                "(p G) b -> p G b", p=N8)
                            nc.sync.dma_start(
                                sout, somaS[32 * j:32 * j + N8, :, :])
                        hT[u] = gather(agin, l, u, bh)

                # ---- output layer (OUT-sharded) ----
                for u in range(nh):
                    c0, c1 = bounds[u]
                    bh = c1 - c0
                    pso = ppd.tile([OS, bh], F32, tag="pd", name=f"pso_u{u}")
                    for kt in range(KT):
                        nc.tensor.matmul(pso[:], woutT[:, kt, :],
                                         hT[u][:, kt, :],
                                         start=(kt == 0), stop=(kt == KT - 1))
                    out_sb = opool.tile([OS, bh], F32, tag="out",
                                        name=f"out_sb_u{u}")
                    nc.scalar.activation(out_sb[:], pso[:], AF.Identity,
                                         bias=b_out[:])
                    nc.sync.dma_start(outT_d[:, c0:c1], out_sb[:])

            for _rep in range(reps):
                one_pass()

    nc.compile()
    return nc


def make_in_maps(x, W_in, b_in, Wd, sd, W_out, b_out, mm_dt=None):
    """Host-side sharding/layout prep. Returns per-core input dicts."""
    del mm_dt
    import ml_dtypes
    ndt = np.dtype(ml_dtypes.bfloat16)
    f32 = np.float32
    x = np.asarray(x, f32)
    W_in = np.asarray(W_in, f32)
    b_in = np.asarray(b_in, f32)
    Wd = np.asarray(Wd, f32)
    sd = np.asarray(sd, f32)
    W_out = np.asarray(W_out, f32)
    b_out = np.asarray(b_out, f32)

    # xT: [k, kt, b] (shared by all cores)
    xT = np.ascontiguousarray(
        x.reshape(B, KT, 128).transpose(2, 1, 0)).astype(ndt)

    # h travels in "storage order": gathered row q (within a 128 block)
    # carries neuron n(q) = 32G+8j+p for q = 32j+4p+G — the order the
    # on-device somaS->agin per-j DMA scan produces. Pre-permute every
    # weight axis that produces or contracts h accordingly.
    qv = np.arange(128)
    perm = 32 * (qv % 4) + 8 * (qv // 32) + (qv % 32) // 4   # n(q)

    # input layer: winT column q of block mt = neuron mt*128+perm[q] over
    # the FULL H, so the redundantly computed hT sits in storage order
    permH = (np.arange(H) // 128) * 128 + np.tile(perm, KT)
    WiP = W_in[permH, :]                                       # [H, IN]
    winT = np.ascontiguousarray(
        WiP.reshape(H, KT, 128).transpose(2, 1, 0)).astype(ndt)
    binP = np.ascontiguousarray(b_in[permH].reshape(KT, 128).T)

    in_maps = []
    for c in range(N_CORES):
        Wd_c = Wd[:, c * HS:(c + 1) * HS, :, :]                # [L, 128, D, H]
        # [l, t, m, d, kt, k] -> [l, k, t, kt, m, d]: partition-major so the
        # device load is one contiguous 32 KB read per partition per layer;
        # contraction rows permuted to storage order (k-axis q <- perm[q])
        wdT = np.ascontiguousarray(
            Wd_c.reshape(L, NDT, N8, D, KT, 128)[..., perm]
            .transpose(0, 5, 1, 4, 2, 3)
        ).reshape(L, 128, NDT * KT * 128).astype(ndt)

        sd_c = sd[:, c * HS:(c + 1) * HS, :]                   # [L, 128, D]
        sd_r = sd_c.reshape(L, NDT, N8, D)                     # [l, t, m, d]
        sdb = np.zeros((128, L, NDT, N8), f32)
        for m in range(N8):
            # partition nd = m*D + d gets sd of neuron m in each tile
            sdb[m * D:(m + 1) * D, :, :, m] = sd_r[:, :, m, :].transpose(2, 0, 1)
        sdb = np.ascontiguousarray(sdb.reshape(128, L * NDT * N8)).astype(ndt)

        Wo = W_out[c * OS:(c + 1) * OS, :]                     # [125, H]
        woutT = np.ascontiguousarray(
            Wo.reshape(OS, KT, 128)[..., perm].transpose(2, 1, 0)).astype(ndt)
        bout_c = np.ascontiguousarray(b_out[c * OS:(c + 1) * OS, None])

        in_maps.append({
            "xT": xT,
            "winT": winT,
            "b_in": binP,
            "wdT": wdT,
            "sdb": sdb,
            "woutT": woutT,
            "b_out": bout_c,
        })
    return in_maps


_CACHE = {}


def get_module(mm_dt=None, wd_bufs=None):
    key = ("v4",)
    if key not in _CACHE:
        _CACHE[key] = build_module()
    return _CACHE[key]


def kernel(x, W_in, b_in, Wd, sd, W_out, b_out):
    """Full-input -> full-output entry point (harness contract)."""
    nc = get_module()
    in_maps = make_in_maps(x, W_in, b_in, Wd, sd, W_out, b_out)
    res = run_bass_kernel_spmd(nc, in_maps, core_ids=list(range(N_CORES)))
    out = np.concatenate([res.results[c]["outT"].T for c in range(N_CORES)],
                         axis=1)
    return np.ascontiguousarray(out.astype(np.float32))
